# revision 1
# baseline (speedup 1.0000x reference)
"""Grouped GEMM (MoE routing) kernel for 8 Trainium2 NeuronCores.

out[off_g : off_g + size_g] = A[off_g : off_g + size_g] @ B[g]   for g in 0..63
A: [524288, 256] f32, B: [64, 256, 256] f32, groups are contiguous row ranges.

Strategy (hardcoded, from the sharding hint "expert-parallel / data-parallel"):
  - Sort groups by tile count (128-row tiles), snake-assign one group per
    (slot, core): slot i takes the groups ranked [8i, 8i+8) — one per core.
    Every core then runs an IDENTICAL static schedule of sum(m_i) tiles,
    where m_i = max tile count in octile i (shorter groups zero-padded).
  - Host packs each core's 8 groups back to back (padded) and pre-transposes
    to AT_core [256, T*128] so the contraction dim is the SBUF partition dim.
  - Device: per-core B (8 experts, 2 MB) stays resident in SBUF; A streams
    in W-tile blocks; per 128-row tile: 2 accumulating matmuls (K=256 split
    across two 128-partition chunks), DVE copy PSUM->SBUF, batched out DMA.
"""

import os
import numpy as np

NCORES = 8
TILE = 128
K = 256
N = 256

# matmul operand dtype on device: "float32" (exact) or "float32r" (fast).
MM_DTYPE = os.environ.get("BASS_GG_DTYPE", "float32r")
W_TILES = int(os.environ.get("BASS_GG_W", "16"))  # tiles per A/out block

LAST_EXEC_NS = None  # set when BASS_GG_TRACE=1

_prog_cache = {}


def _schedule(sizes):
    """sizes -> (slots [nslot, NCORES] group ids, m [nslot] tile budgets)."""
    sizes = np.asarray(sizes, dtype=np.int64)
    g = sizes.shape[0]
    pad_groups = (-g) % NCORES
    if pad_groups:
        sizes = np.concatenate([sizes, np.zeros(pad_groups, np.int64)])
    ntiles = (sizes + TILE - 1) // TILE
    order = np.argsort(-ntiles, kind="stable")
    nslot = len(sizes) // NCORES
    slots = order.reshape(nslot, NCORES)
    m = ntiles[slots[:, 0]].astype(np.int64)
    keep = m > 0
    return slots[keep], m[keep]


def _build_program(m_list, dtype_name, w_tiles):
    import concourse.tile as tile
    from concourse import bacc, mybir

    DT = getattr(mybir.dt, dtype_name)
    R = len(m_list)
    T = int(sum(m_list))

    nc = bacc.Bacc(
        "TRN2",
        target_bir_lowering=False,
        debug=False,
        enable_asserts=False,
        num_devices=NCORES,
    )
    AT = nc.dram_tensor("AT", [K, T * TILE], DT, kind="ExternalInput").ap()
    BW = nc.dram_tensor("BW", [R, 2, 128, N], DT, kind="ExternalInput").ap()
    OUT = nc.dram_tensor("OUT", [T * TILE, N], mybir.dt.float32, kind="ExternalOutput").ap()

    slot_of = []
    for i, mi in enumerate(m_list):
        slot_of += [i] * int(mi)

    with tile.TileContext(nc) as tc:
        with tc.tile_pool(name="bpool", bufs=1) as bpool, \
             tc.tile_pool(name="apool", bufs=3) as apool, \
             tc.tile_pool(name="opool", bufs=3) as opool, \
             tc.tile_pool(name="psum", bufs=8, space="PSUM") as pspool:
            b_sb = bpool.tile([128, R, 2, N], DT)
            nc.sync.dma_start(out=b_sb, in_=BW.rearrange("r j p n -> p r j n"))
            OUTv = OUT.rearrange("(t p) n -> p t n", p=TILE)
            nblk = (T + w_tiles - 1) // w_tiles
            for blk in range(nblk):
                t0 = blk * w_tiles
                w = min(w_tiles, T - t0)
                a0 = apool.tile([128, w_tiles * TILE], DT, tag="a0")
                a1 = apool.tile([128, w_tiles * TILE], DT, tag="a1")
                nc.sync.dma_start(
                    out=a0[:, : w * TILE], in_=AT[0:128, t0 * TILE : (t0 + w) * TILE]
                )
                nc.sync.dma_start(
                    out=a1[:, : w * TILE], in_=AT[128:256, t0 * TILE : (t0 + w) * TILE]
                )
                ob = opool.tile([128, w_tiles, N], mybir.dt.float32, tag="ob")
                for t in range(w):
                    s = slot_of[t0 + t]
                    ps = pspool.tile([128, N], mybir.dt.float32)
                    nc.tensor.matmul(
                        ps,
                        lhsT=a0[:, t * TILE : (t + 1) * TILE],
                        rhs=b_sb[:, s, 0, :],
                        start=True,
                        stop=False,
                    )
                    nc.tensor.matmul(
                        ps,
                        lhsT=a1[:, t * TILE : (t + 1) * TILE],
                        rhs=b_sb[:, s, 1, :],
                        start=False,
                        stop=True,
                    )
                    nc.vector.tensor_copy(out=ob[:, t, :], in_=ps)
                nc.scalar.dma_start(out=OUTv[:, t0 : t0 + w, :], in_=ob[:, :w, :])
    nc.compile()
    return nc


def _get_program(m_key, dtype_name, w_tiles):
    key = (m_key, dtype_name, w_tiles)
    if key not in _prog_cache:
        _prog_cache[key] = _build_program(list(m_key), dtype_name, w_tiles)
    return _prog_cache[key]


def kernel(A, B, batch_sizes, batch_offsets, batch_padded_offsets):
    global LAST_EXEC_NS
    from concourse.bass_utils import run_bass_kernel_spmd

    A = np.asarray(A, dtype=np.float32)
    B = np.asarray(B, dtype=np.float32)
    sizes = np.asarray(batch_sizes, dtype=np.int64)
    offsets = np.asarray(batch_offsets, dtype=np.int64)

    M = A.shape[0]
    slots, m = _schedule(sizes)
    T = int(m.sum())
    starts = np.concatenate([[0], np.cumsum(m)[:-1]])  # slot start, in tiles

    nc = _get_program(tuple(int(x) for x in m), MM_DTYPE, W_TILES)

    in_maps = []
    for c in range(NCORES):
        at = np.zeros((K, T * TILE), dtype=np.float32)
        bw = np.zeros((len(m), 2, 128, N), dtype=np.float32)
        for i in range(len(m)):
            g = int(slots[i, c])
            off, sz = int(offsets[g]), int(sizes[g])
            dst = int(starts[i]) * TILE
            if sz > 0:
                at[:, dst : dst + sz] = A[off : off + sz].T
            bw[i] = B[g].reshape(2, 128, N)
        in_maps.append({"AT": at, "BW": bw})

    trace = bool(int(os.environ.get("BASS_GG_TRACE", "0")))
    res = run_bass_kernel_spmd(
        nc, in_maps, core_ids=list(range(NCORES)), trace=trace
    )
    LAST_EXEC_NS = res.exec_time_ns

    out = np.zeros((M, N), dtype=np.float32)
    for c in range(NCORES):
        oc = res.results[c]["OUT"]
        for i in range(len(m)):
            g = int(slots[i, c])
            off, sz = int(offsets[g]), int(sizes[g])
            src = int(starts[i]) * TILE
            if sz > 0:
                out[off : off + sz] += oc[src : src + sz]
    return out



# revision 2
# speedup vs baseline: 2.3427x; 2.3427x over previous
"""Grouped GEMM (MoE routing) kernel for 8 Trainium2 NeuronCores.

out[off_g : off_g + size_g] = A[off_g : off_g + size_g] @ B[g]   for g in 0..63
A: [524288, 256] f32, B: [64, 256, 256] f32, groups are contiguous row ranges.

Strategy (hardcoded, from the sharding hint "expert-parallel"):
  - Sort groups by tile count (128-row tiles), snake-assign one group per
    (slot, core): every core runs an IDENTICAL static schedule of T tiles.
  - bf16 operands + bf16 output (accumulation stays f32 in PSUM): halves
    HBM traffic vs f32; rel err ~4e-3, well inside the 2e-2 gate.
  - Host packs each core's groups back to back, pre-transposed to
    AT [256, T*128] bf16 so the contraction dim is the SBUF partition dim.
  - Device computes the TRANSPOSED output OUTT [256, T*128] so every DMA
    touches 8KB-contiguous per-partition lines (the f32 row-major output
    layout only allowed 1KB strided lines, which is packet-rate bound).
  - Per 32-tile block, per expert segment: for each output half h (128 of
    the 256 N columns) load the stationary weight once per K-chunk and
    stream quad-tile matmuls (moving free dim 512 = one full PSUM bank),
    then cast-copy PSUM->SBUF on the vector (h=0) / scalar (h=1) engine.
"""

import os
import numpy as np

NCORES = 8
TILE = 128
K = 256
N = 256
QUAD = 4  # tiles per PSUM bank (4*128 = 512 f32 = 2KB = one bank)

W_TILES = int(os.environ.get("BASS_GG_W", "32"))  # tiles per A/out block

LAST_EXEC_NS = None  # set when BASS_GG_TRACE=1

_prog_cache = {}


def _schedule(sizes):
    """sizes -> (slots [nslot, NCORES] group ids, m [nslot] tile budgets).

    Every core c executes slots[i, c] in slot i, which is budgeted for m[i]
    tiles (shorter groups zero-padded). Picks the better of snake assignment
    and LPT bin packing (both minimize sum-of-column-maxima T = sum(m)).
    """
    sizes = np.asarray(sizes, dtype=np.int64)
    g = sizes.shape[0]
    pad_groups = (-g) % NCORES
    if pad_groups:
        sizes = np.concatenate([sizes, np.zeros(pad_groups, np.int64)])
    ntiles = (sizes + TILE - 1) // TILE
    order = np.argsort(-ntiles, kind="stable")
    nslot = len(sizes) // NCORES

    # snake: rank 8i..8i+7 -> slot i
    snake = order.reshape(nslot, NCORES)
    snake_m = ntiles[snake[:, 0]].astype(np.int64)

    # LPT: greedily assign (desc) to least-loaded core, sort each core desc
    loads = np.zeros(NCORES, dtype=np.int64)
    percore = [[] for _ in range(NCORES)]
    for gid in order:
        c = int(np.argmin(loads))
        if len(percore[c]) >= nslot:
            c = int(np.argmin(np.where(
                np.array([len(p) for p in percore]) < nslot, loads, np.iinfo(np.int64).max)))
        percore[c].append(int(gid))
        loads[c] += ntiles[gid]
    lpt = np.zeros((nslot, NCORES), dtype=np.int64)
    for c in range(NCORES):
        lpt[:, c] = sorted(percore[c], key=lambda x: -ntiles[x])
    lpt_m = ntiles[lpt].max(axis=1).astype(np.int64)

    if lpt_m.sum() < snake_m.sum():
        slots, m = lpt, lpt_m
    else:
        slots, m = snake, snake_m
    keep = m > 0
    return slots[keep], m[keep]


def _build_program(m_list, w_tiles):
    import concourse.tile as tile
    from concourse import bacc, mybir

    BF16 = mybir.dt.bfloat16
    F32 = mybir.dt.float32
    R = len(m_list)
    T = int(sum(m_list))

    nc = bacc.Bacc(
        "TRN2",
        target_bir_lowering=False,
        debug=False,
        enable_asserts=False,
        num_devices=NCORES,
    )
    AT = nc.dram_tensor("AT", [K, T * TILE], BF16, kind="ExternalInput").ap()
    BW = nc.dram_tensor("BW", [128, R, 2, 2, 128], BF16, kind="ExternalInput").ap()
    OUTT = nc.dram_tensor("OUTT", [N, T * TILE], BF16, kind="ExternalOutput").ap()

    slot_of = []
    for i, mi in enumerate(m_list):
        slot_of += [i] * int(mi)

    with tile.TileContext(nc) as tc:
        with tc.tile_pool(name="bpool", bufs=1) as bpool, \
             tc.tile_pool(name="apool", bufs=3) as apool, \
             tc.tile_pool(name="opool", bufs=3) as opool, \
             tc.tile_pool(name="psum", bufs=8, space="PSUM") as pspool:
            b_sb = bpool.tile([128, R, 2, 2, 128], BF16)
            nc.sync.dma_start(out=b_sb, in_=BW)
            nblk = (T + w_tiles - 1) // w_tiles
            for blk in range(nblk):
                t0 = blk * w_tiles
                w = min(w_tiles, T - t0)
                a0 = apool.tile([128, w_tiles * TILE], BF16, tag="a0")
                a1 = apool.tile([128, w_tiles * TILE], BF16, tag="a1")
                nc.sync.dma_start(
                    out=a0[:, : w * TILE], in_=AT[0:128, t0 * TILE : (t0 + w) * TILE]
                )
                nc.sync.dma_start(
                    out=a1[:, : w * TILE], in_=AT[128:256, t0 * TILE : (t0 + w) * TILE]
                )
                ob = opool.tile([128, 2, w_tiles * TILE], BF16, tag="ob")

                # segments of equal expert slot within this block
                segs = []
                t = 0
                while t < w:
                    s = slot_of[t0 + t]
                    r = 1
                    while t + r < w and slot_of[t0 + t + r] == s:
                        r += 1
                    segs.append((t, r, s))
                    t += r

                for (ts, rlen, s) in segs:
                    # chunks of up to 8 quads (uses all 8 PSUM banks)
                    for c0 in range(ts, ts + rlen, 8 * QUAD):
                        clen = min(8 * QUAD, ts + rlen - c0)
                        quads = []
                        q = 0
                        while q < clen:
                            ql = min(QUAD, clen - q)
                            quads.append((c0 + q, ql))
                            q += ql
                        for h in range(2):
                            pss = []
                            for (qt, ql) in quads:
                                ps = pspool.tile([128, QUAD * TILE], F32)
                                pss.append(ps)
                            for j, aj in ((0, a0), (1, a1)):
                                for (qt, ql), ps in zip(quads, pss):
                                    nc.tensor.matmul(
                                        ps[:, : ql * TILE],
                                        lhsT=b_sb[:, s, j, h, :],
                                        rhs=aj[:, qt * TILE : (qt + ql) * TILE],
                                        start=(j == 0),
                                        stop=(j == 1),
                                    )
                            eng = nc.vector.tensor_copy if h == 0 else nc.scalar.copy
                            for (qt, ql), ps in zip(quads, pss):
                                eng(
                                    out=ob[:, h, qt * TILE : (qt + ql) * TILE],
                                    in_=ps[:, : ql * TILE],
                                )
                for h in range(2):
                    nc.gpsimd.dma_start(
                        out=OUTT[h * 128 : (h + 1) * 128, t0 * TILE : (t0 + w) * TILE],
                        in_=ob[:, h, : w * TILE],
                    )
    nc.compile()
    return nc


def _get_program(m_key, w_tiles):
    key = (m_key, w_tiles)
    if key not in _prog_cache:
        _prog_cache[key] = _build_program(list(m_key), w_tiles)
    return _prog_cache[key]


def kernel(A, B, batch_sizes, batch_offsets, batch_padded_offsets):
    global LAST_EXEC_NS
    import ml_dtypes
    from concourse.bass_utils import run_bass_kernel_spmd

    bf16 = ml_dtypes.bfloat16
    A = np.asarray(A, dtype=np.float32)
    B = np.asarray(B, dtype=np.float32)
    sizes = np.asarray(batch_sizes, dtype=np.int64)
    offsets = np.asarray(batch_offsets, dtype=np.int64)

    M = A.shape[0]
    slots, m = _schedule(sizes)
    T = int(m.sum())
    starts = np.concatenate([[0], np.cumsum(m)[:-1]])  # slot start, in tiles

    nc = _get_program(tuple(int(x) for x in m), W_TILES)

    ATfull = np.ascontiguousarray(A.astype(bf16).T)  # [K, M]
    Bbf = B.astype(bf16)  # [G, K, N]

    in_maps = []
    for c in range(NCORES):
        at = np.zeros((K, T * TILE), dtype=bf16)
        bw = np.zeros((128, len(m), 2, 2, 128), dtype=bf16)
        for i in range(len(m)):
            g = int(slots[i, c])
            off, sz = int(offsets[g]), int(sizes[g])
            dst = int(starts[i]) * TILE
            if sz > 0:
                at[:, dst : dst + sz] = ATfull[:, off : off + sz]
            # bw[p, i, j, h, n] = B[g, j*128+p, h*128+n]
            bw[:, i] = Bbf[g].reshape(2, 128, 2, 128).transpose(1, 0, 2, 3)
        in_maps.append({"AT": at, "BW": bw})

    trace = bool(int(os.environ.get("BASS_GG_TRACE", "0")))
    res = run_bass_kernel_spmd(
        nc, in_maps, core_ids=list(range(NCORES)), trace=trace
    )
    LAST_EXEC_NS = res.exec_time_ns

    outT = np.zeros((N, M), dtype=np.float32)
    for c in range(NCORES):
        oc = res.results[c]["OUTT"]
        for i in range(len(m)):
            g = int(slots[i, c])
            off, sz = int(offsets[g]), int(sizes[g])
            src = int(starts[i]) * TILE
            if sz > 0:
                outT[:, off : off + sz] = oc[:, src : src + sz]
    return outT.T


# revision 4
# speedup vs baseline: 2.5922x; 1.1065x over previous
"""Grouped GEMM (MoE routing) kernel for 8 Trainium2 NeuronCores.

out[off_g : off_g + size_g] = A[off_g : off_g + size_g] @ B[g]   for g in 0..63
A: [524288, 256] f32, B: [64, 256, 256] f32, groups are contiguous row ranges.

Strategy (hardcoded, from the sharding hint "expert-parallel"):
  - Sort groups by tile count (128-row tiles), snake-assign one group per
    (slot, core): every core runs an IDENTICAL static schedule of T tiles.
  - bf16 operands + bf16 output (accumulation stays f32 in PSUM): halves
    HBM traffic vs f32; rel err ~4e-3, well inside the 2e-2 gate.
  - Host packs each core's groups back to back, pre-transposed to
    AT [256, T*128] bf16 so the contraction dim is the SBUF partition dim.
  - Device computes the TRANSPOSED output OUTT [256, T*128] so every DMA
    touches 8KB-contiguous per-partition lines (the f32 row-major output
    layout only allowed 1KB strided lines, which is packet-rate bound).
  - Per 32-tile block, per expert segment: for each output half h (128 of
    the 256 N columns) load the stationary weight once per K-chunk and
    stream quad-tile matmuls (moving free dim 512 = one full PSUM bank),
    then cast-copy PSUM->SBUF on the vector (h=0) / scalar (h=1) engine.
"""

import os
import numpy as np

NCORES = 8
TILE = 128
K = 256
N = 256
QUAD = 4  # tiles per PSUM bank (4*128 = 512 f32 = 2KB = one bank)

W_TILES = int(os.environ.get("BASS_GG_W", "32"))  # tiles per A/out block

LAST_EXEC_NS = None  # set when BASS_GG_TRACE=1

_prog_cache = {}


def _schedule(sizes):
    """sizes -> (slots [nslot, NCORES] group ids, m [nslot] tile budgets).

    Every core c executes slots[i, c] in slot i, which is budgeted for m[i]
    tiles (shorter groups zero-padded). Picks the better of snake assignment
    and LPT bin packing (both minimize sum-of-column-maxima T = sum(m)).
    """
    sizes = np.asarray(sizes, dtype=np.int64)
    g = sizes.shape[0]
    pad_groups = (-g) % NCORES
    if pad_groups:
        sizes = np.concatenate([sizes, np.zeros(pad_groups, np.int64)])
    ntiles = (sizes + TILE - 1) // TILE
    order = np.argsort(-ntiles, kind="stable")
    nslot = len(sizes) // NCORES

    # snake: rank 8i..8i+7 -> slot i
    snake = order.reshape(nslot, NCORES)
    snake_m = ntiles[snake[:, 0]].astype(np.int64)

    # LPT: greedily assign (desc) to least-loaded core, sort each core desc
    loads = np.zeros(NCORES, dtype=np.int64)
    percore = [[] for _ in range(NCORES)]
    for gid in order:
        c = int(np.argmin(loads))
        if len(percore[c]) >= nslot:
            c = int(np.argmin(np.where(
                np.array([len(p) for p in percore]) < nslot, loads, np.iinfo(np.int64).max)))
        percore[c].append(int(gid))
        loads[c] += ntiles[gid]
    lpt = np.zeros((nslot, NCORES), dtype=np.int64)
    for c in range(NCORES):
        lpt[:, c] = sorted(percore[c], key=lambda x: -ntiles[x])
    lpt_m = ntiles[lpt].max(axis=1).astype(np.int64)

    if lpt_m.sum() < snake_m.sum():
        slots, m = lpt, lpt_m
    else:
        slots, m = snake, snake_m
    keep = m > 0
    return slots[keep], m[keep]


def _build_program(m_list, w_tiles):
    import concourse.tile as tile
    from concourse import bacc, mybir

    BF16 = mybir.dt.bfloat16
    F32 = mybir.dt.float32
    R = len(m_list)
    T = int(sum(m_list))

    nc = bacc.Bacc(
        "TRN2",
        target_bir_lowering=False,
        debug=False,
        enable_asserts=False,
        num_devices=NCORES,
    )
    AT = nc.dram_tensor("AT", [K, T * TILE], BF16, kind="ExternalInput").ap()
    BW = nc.dram_tensor("BW", [128, R, 2, 2, 128], BF16, kind="ExternalInput").ap()
    OUTT = nc.dram_tensor("OUTT", [N, T * TILE], BF16, kind="ExternalOutput").ap()

    slot_of = []
    for i, mi in enumerate(m_list):
        slot_of += [i] * int(mi)

    # block sizes: taper at both ends to shrink pipeline fill/drain
    blocks = []
    t0 = 0
    lead = [8, 8, 16]
    tail = [16, 8, 8]
    mid = T - sum(lead) - sum(tail)
    for w in lead:
        blocks.append((t0, w))
        t0 += w
    while mid > 0:
        w = min(w_tiles, mid)
        blocks.append((t0, w))
        t0 += w
        mid -= w
    for w in tail:
        blocks.append((t0, w))
        t0 += w
    assert t0 == T

    LOOKAHEAD = 3  # emit load triggers this many blocks ahead of compute

    with tile.TileContext(nc) as tc:
        with tc.tile_pool(name="bpool", bufs=1) as bpool, \
             tc.tile_pool(name="apool", bufs=4) as apool, \
             tc.tile_pool(name="opool", bufs=3) as opool, \
             tc.tile_pool(name="psum", bufs=8, space="PSUM") as pspool:
            b_sb = bpool.tile([128, R, 2, 2, 128], BF16)
            nc.gpsimd.dma_start(out=b_sb, in_=BW)

            abufs = {}

            def emit_loads(bi):
                t0, w = blocks[bi]
                a0 = apool.tile([128, w_tiles * TILE], BF16, tag="a0")
                a1 = apool.tile([128, w_tiles * TILE], BF16, tag="a1")
                nc.sync.dma_start(
                    out=a0[:, : w * TILE], in_=AT[0:128, t0 * TILE : (t0 + w) * TILE]
                )
                nc.scalar.dma_start(
                    out=a1[:, : w * TILE], in_=AT[128:256, t0 * TILE : (t0 + w) * TILE]
                )
                abufs[bi] = (a0, a1)

            def emit_compute(bi):
                t0, w = blocks[bi]
                a0, a1 = abufs.pop(bi)
                ob = opool.tile([128, 2, w_tiles * TILE], BF16, tag="ob")

                # segments of equal expert slot within this block
                segs = []
                t = 0
                while t < w:
                    s = slot_of[t0 + t]
                    r = 1
                    while t + r < w and slot_of[t0 + t + r] == s:
                        r += 1
                    segs.append((t, r, s))
                    t += r

                for (ts, rlen, s) in segs:
                    # chunks of up to 8 quads (uses all 8 PSUM banks)
                    for c0 in range(ts, ts + rlen, 8 * QUAD):
                        clen = min(8 * QUAD, ts + rlen - c0)
                        quads = []
                        q = 0
                        while q < clen:
                            ql = min(QUAD, clen - q)
                            quads.append((c0 + q, ql))
                            q += ql
                        for h in range(2):
                            pss = []
                            for (qt, ql) in quads:
                                ps = pspool.tile([128, QUAD * TILE], F32)
                                pss.append(ps)
                            for j, aj in ((0, a0), (1, a1)):
                                for (qt, ql), ps in zip(quads, pss):
                                    nc.tensor.matmul(
                                        ps[:, : ql * TILE],
                                        lhsT=b_sb[:, s, j, h, :],
                                        rhs=aj[:, qt * TILE : (qt + ql) * TILE],
                                        start=(j == 0),
                                        stop=(j == 1),
                                    )
                            eng = nc.vector.tensor_copy if h == 0 else nc.scalar.copy
                            for (qt, ql), ps in zip(quads, pss):
                                eng(
                                    out=ob[:, h, qt * TILE : (qt + ql) * TILE],
                                    in_=ps[:, : ql * TILE],
                                )
                for h, deng in ((0, nc.gpsimd), (1, nc.sync)):
                    deng.dma_start(
                        out=OUTT[h * 128 : (h + 1) * 128, t0 * TILE : (t0 + w) * TILE],
                        in_=ob[:, h, : w * TILE],
                    )

            nblk = len(blocks)
            for bi in range(nblk + LOOKAHEAD):
                if bi < nblk:
                    emit_loads(bi)
                if bi >= LOOKAHEAD:
                    emit_compute(bi - LOOKAHEAD)
    nc.compile()
    return nc


def _get_program(m_key, w_tiles):
    key = (m_key, w_tiles)
    if key not in _prog_cache:
        _prog_cache[key] = _build_program(list(m_key), w_tiles)
    return _prog_cache[key]


def kernel(A, B, batch_sizes, batch_offsets, batch_padded_offsets):
    global LAST_EXEC_NS
    import ml_dtypes
    from concourse.bass_utils import run_bass_kernel_spmd

    bf16 = ml_dtypes.bfloat16
    A = np.asarray(A, dtype=np.float32)
    B = np.asarray(B, dtype=np.float32)
    sizes = np.asarray(batch_sizes, dtype=np.int64)
    offsets = np.asarray(batch_offsets, dtype=np.int64)

    M = A.shape[0]
    slots, m = _schedule(sizes)
    T = int(m.sum())
    starts = np.concatenate([[0], np.cumsum(m)[:-1]])  # slot start, in tiles

    nc = _get_program(tuple(int(x) for x in m), W_TILES)

    ATfull = np.ascontiguousarray(A.astype(bf16).T)  # [K, M]
    Bbf = B.astype(bf16)  # [G, K, N]

    in_maps = []
    for c in range(NCORES):
        at = np.zeros((K, T * TILE), dtype=bf16)
        bw = np.zeros((128, len(m), 2, 2, 128), dtype=bf16)
        for i in range(len(m)):
            g = int(slots[i, c])
            off, sz = int(offsets[g]), int(sizes[g])
            dst = int(starts[i]) * TILE
            if sz > 0:
                at[:, dst : dst + sz] = ATfull[:, off : off + sz]
            # bw[p, i, j, h, n] = B[g, j*128+p, h*128+n]
            bw[:, i] = Bbf[g].reshape(2, 128, 2, 128).transpose(1, 0, 2, 3)
        in_maps.append({"AT": at, "BW": bw})

    trace = bool(int(os.environ.get("BASS_GG_TRACE", "0")))
    res = run_bass_kernel_spmd(
        nc, in_maps, core_ids=list(range(NCORES)), trace=trace
    )
    LAST_EXEC_NS = res.exec_time_ns

    outT = np.zeros((N, M), dtype=np.float32)
    for c in range(NCORES):
        oc = res.results[c]["OUTT"]
        for i in range(len(m)):
            g = int(slots[i, c])
            off, sz = int(offsets[g]), int(sizes[g])
            src = int(starts[i]) * TILE
            if sz > 0:
                outT[:, off : off + sz] = oc[:, src : src + sz]
    return outT.T
